# revision 15
# baseline (speedup 1.0000x reference)
"""Fused LayerNorm->MHA(multi-query)->LayerNorm kernel for TRN2, 8 cores SPMD.

Problem shapes (hardcoded):
  x:        [4, 2048, 512] f32
  attn_bias:[8, 2048, 2048] f32   (shared across batch)
  w_q:      [512, 512], w_kv: [512, 128], w_out: [512, 512]
  g_in, g_out: [512]
  out:      [4, 2048, 512] f32

Sharding: 8 cores = (batch b in 0..3) x (query-half ih in 0..1).
Each core computes the full pipeline for one batch and 1024 query rows.

Restructured v2 (vs the 274us baseline):
  - A@V runs "swapped": the attention tile e[j, i-block] is the PE
    stationary operand and v (64 cols + ones) streams as the moving
    operand -> 65 cols/matmul instead of 512, and the output lands in
    [i, dh] layout with the softmax denominator as a PER-PARTITION
    column, so normalization is one tensor_scalar per tile instead of
    the DRAM scatter/gather reciprocal dance.
  - attn_bias is streamed as exp(bias) fp16 with a host-side layout
    giving 16KiB contiguous HBM lines per partition (128 descriptors
    per 2MiB chunk instead of 512 per 1MiB) -> DMA runs at wire speed.
  - attention emission is interleaved with the tail of LayerNorm /
    projections for j rows 1024:2048 so compute starts after only half
    of phase 1, and the final out-projection consumes PE-transposed
    [dh, i] tiles built in phase 4.
"""

import sys

sys.path.insert(0, "/opt/trn_rl_repo")

import numpy as np
from contextlib import ExitStack

import concourse.bass as bass
import concourse.tile as tile
from concourse import bacc
from concourse import mybir
from concourse.masks import make_identity

B, N, DIM = 4, 2048, 512
HEADS, DH = 8, 64
INNER = HEADS * DH  # 512
EPS = 1e-5
SCALE = DH ** -0.5
NCORES = 8
IH = N // 2  # 1024 query rows per core
P = 128

NT = N // P      # 16 row tiles of x / j tiles
DT = DIM // P    # 4 d tiles
CT = INNER // P  # 4 c tiles (head pairs)
ICH = IH // 512  # 2 i chunks of 512
JT = N // P      # 16 j tiles
JPAIR = 4        # j tiles per bias DMA chunk
NHP = HEADS // 2
ITB = IH // P    # 8 i blocks of 128

F32 = mybir.dt.float32
F16 = mybir.dt.float16

BIAS_DT = mybir.dt.float16


def build_bass():
    nc = bacc.Bacc("TRN2")
    x_d = nc.dram_tensor("x", [N, DIM], F32, kind="ExternalInput")
    # [h, jp, p, t*IH] : per (h, jp) chunk each partition reads one
    # contiguous 8 KiB line
    bias_d = nc.dram_tensor(
        "biasT", [HEADS, JT // JPAIR, P, JPAIR * IH], BIAS_DT,
        kind="ExternalInput")
    wq_d = nc.dram_tensor("wq", [DIM, INNER], F16, kind="ExternalInput")
    wkv_d = nc.dram_tensor("wkv", [DIM, 2 * DH], F16, kind="ExternalInput")
    wout_d = nc.dram_tensor("wout", [INNER, DIM], F16, kind="ExternalInput")
    out_d = nc.dram_tensor("out", [IH, DIM], F32, kind="ExternalOutput")

    with tile.TileContext(nc) as tc:
        _body(tc, x_d, bias_d, wq_d, wkv_d, wout_d, out_d)
    nc.compile()
    return nc


def _body(tc, x_d, bias_d, wq_d, wkv_d, wout_d, out_d):
    nc = tc.nc
    ctx = ExitStack()
    with ctx:
        persist = ctx.enter_context(tc.tile_pool(name="persist", bufs=1))

        identity = persist.tile([P, P], F16, name="identity")
        eps_t = persist.tile([P, 1], F32, name="eps")

        # weights
        wq_sb = [persist.tile([P, INNER], F16, name=f"wq{d}") for d in range(DT)]
        wkv_sb = [persist.tile([P, 2 * DH], F16, name=f"wkv{d}") for d in range(DT)]
        wout_sb = [persist.tile([P, DIM], F16, name=f"wout{t}")
                   for t in range(CT)]

        ebp = ctx.enter_context(tc.tile_pool(name="bias", bufs=3))

        def eb_dma(h, jp):
            t = ebp.tile([P, JPAIR, IH], BIAS_DT, name="eb")
            nc.sync.dma_start(
                out=t,
                in_=bias_d[h, jp].rearrange("p (t i) -> p t i", t=JPAIR))
            return t

        # persistent on-chip tensors
        xnT = [persist.tile([P, N], F16, name=f"xnT{d}") for d in range(DT)]
        kT2 = persist.tile([P, N], F16, name="kT2")
        vp = [persist.tile([P, DH + 1], F16, name=f"vp{j}") for j in range(JT)]
        qT = [persist.tile([P, IH], F16, name=f"qT{t}") for t in range(CT)]
        # attention output, normalized, [i-part, (it, hh*dh)] per head pair
        ao_sb = [persist.tile([P, ITB, P], F16, name=f"ao{t}") for t in range(NHP)]
        # transposed [(hh, dh), i] per head pair
        aoT = [persist.tile([P, IH], F16, name=f"aoT{t}") for t in range(NHP)]

        ln = ctx.enter_context(tc.tile_pool(name="ln", bufs=3))
        proj = ctx.enter_context(tc.tile_pool(name="proj", bufs=2))
        lnps_box = [None]
        projps_box = [None]
        x_tiles_box = [None]

        def ln_group(g, apply_eng):
            lnps = lnps_box[0]
            x_tiles = x_tiles_box[0]
            """LayerNorm 4 x-tiles of group g and transpose into xnT."""
            for nt in range(4 * g, 4 * g + 4):
                x_t = x_tiles[nt]
                stats = ln.tile([P, 6], F32, name="stats")
                nc.vector.bn_stats(out=stats, in_=x_t)
                mv = ln.tile([P, 2], F32, name="mv")
                nc.vector.bn_aggr(out=mv, in_=stats)
                rstd = ln.tile([P, 1], F32, name="rstd")
                nc.scalar.activation(
                    out=rstd, in_=mv[:, 1:2],
                    func=mybir.ActivationFunctionType.Sqrt,
                    bias=eps_t, scale=1.0)
                nc.vector.reciprocal(out=rstd, in_=rstd)
                negmr = ln.tile([P, 1], F32, name="negmr")
                nc.vector.tensor_scalar(
                    out=negmr, in0=mv[:, 0:1], scalar1=rstd, scalar2=-1.0,
                    op0=mybir.AluOpType.mult, op1=mybir.AluOpType.mult)
                xn_t = ln.tile([P, DIM], F16, name="xn_t")
                if apply_eng == "act":
                    nc.scalar.activation(
                        out=xn_t, in_=x_t,
                        func=mybir.ActivationFunctionType.Identity,
                        bias=negmr, scale=rstd)
                else:
                    nc.vector.tensor_scalar(
                        out=xn_t, in0=x_t, scalar1=rstd, scalar2=negmr,
                        op0=mybir.AluOpType.mult, op1=mybir.AluOpType.add)
                for d in range(DT):
                    ps = lnps.tile([P, P], F16, name="tps")
                    nc.tensor.transpose(ps, xn_t[:, d * P:(d + 1) * P], identity)
                    nc.vector.tensor_copy(
                        out=xnT[d][:, nt * P:(nt + 1) * P], in_=ps)

        def kv_group(g):
            """kv projection + kT2 + v row tiles for j chunk g (512 rows)."""
            projps = projps_box[0]
            lo = g * 512
            kvT = proj.tile([P, 512], F16, name="kvT")
            ps = projps.tile([P, 512], F32, name="kvps")
            for d in range(DT):
                nc.tensor.matmul(
                    ps, wkv_sb[d], xnT[d][:, lo:lo + 512],
                    start=(d == 0), stop=(d == DT - 1))
            nc.vector.tensor_copy(out=kvT, in_=ps)
            # k rows (partitions 64:128, host-swapped [v, k]) into both kT2
            # halves; partition-shifting copy goes through DMA
            nc.vector.tensor_copy(out=kT2[DH:2 * DH, lo:lo + 512],
                                  in_=kvT[DH:2 * DH, :])
            nc.sync.dma_start(out=kT2[0:DH, lo:lo + 512], in_=kvT[DH:2 * DH, :])
            # v row tiles with ones column
            for jj in range(4):
                j = 4 * g + jj
                vps = projps.tile([P, DH], F16, name="vps")
                nc.tensor.transpose(
                    vps, kvT[0:DH, jj * P:(jj + 1) * P], identity[0:DH, 0:DH])
                nc.vector.tensor_copy(out=vp[j][:, 0:DH], in_=vps)
                nc.vector.memset(vp[j][:, DH:DH + 1], 1.0)

        def q_group(g):
            """q projection for i chunk g (512 rows)."""
            projps = projps_box[0]
            lo = g * 512
            for t in range(CT):
                ps = projps.tile([P, 512], F32, name="qps")
                for d in range(DT):
                    nc.tensor.matmul(
                        ps, wq_sb[d][:, t * P:(t + 1) * P],
                        xnT[d][:, lo:lo + 512],
                        start=(d == 0), stop=(d == DT - 1))
                nc.vector.tensor_copy(out=qT[t][:, lo:lo + 512], in_=ps)

        # Phase 1+2: LayerNorm + projections. Local query rows (0:1024)
        # first; bias chunks for the first head interleave onto the DMA ring
        # before x groups 2-3 so attention can start as soon as qT is ready.
        eb_pre = []
        with tc.tile_pool(name="xload", bufs=1) as xload, \
             tc.tile_pool(name="lnps", bufs=2, space="PSUM") as lnps, \
             tc.tile_pool(name="projps", bufs=2, space="PSUM") as projps:
            x_big = [xload.tile([P, 4, DIM], F32, name=f"x{g}")
                     for g in range(4)]
            for g in range(2):
                nc.sync.dma_start(
                    out=x_big[g],
                    in_=x_d[g * 4 * P:(g + 1) * 4 * P, :]
                    .rearrange("(t p) d -> p t d", p=P))
            make_identity(nc, identity)
            nc.vector.memset(eps_t, EPS)
            for d in range(DT):
                nc.sync.dma_start(out=wq_sb[d], in_=wq_d[d * P:(d + 1) * P, :])
                nc.sync.dma_start(out=wkv_sb[d], in_=wkv_d[d * P:(d + 1) * P, :])
            for t in range(CT):
                nc.sync.dma_start(out=wout_sb[t],
                                  in_=wout_d[t * P:(t + 1) * P, :])
            eb_pre.append(eb_dma(0, 0))
            eb_pre.append(eb_dma(0, 1))
            for g in range(2, 4):
                nc.sync.dma_start(
                    out=x_big[g],
                    in_=x_d[g * 4 * P:(g + 1) * 4 * P, :]
                    .rearrange("(t p) d -> p t d", p=P))
            x_tiles_box[0] = [x_big[nt // 4][:, nt % 4, :] for nt in range(NT)]
            lnps_box[0] = lnps
            projps_box[0] = projps
            for g in range(2):
                ln_group(g, "act")
                kv_group(g)
                q_group(g)
            for g in range(2, 4):
                ln_group(g, "dve")
                kv_group(g)

        # ---- attention: 8 stages (head-serial); A@V runs pipelined one
        # stage behind so each accumulation group owns one PSUM bank ----
        ep = ctx.enter_context(tc.tile_pool(name="ep", bufs=1))
        recp = ctx.enter_context(tc.tile_pool(name="recp", bufs=4))
        e_tiles = [[ep.tile([P, ICH, 512], F16, name=f"e{par}_{j}")
                    for j in range(JT)] for par in range(2)]
        attn_ps = ExitStack()
        qkps = attn_ps.enter_context(
            tc.tile_pool(name="qkps", bufs=2, space="PSUM"))
        avps = attn_ps.enter_context(
            tc.tile_pool(name="avps", bufs=3, space="PSUM"))

        def av_group(s, it):
            """A@V for stage s (head s), i block it: one 16-matmul
            accumulation group in a single PSUM bank, then normalize."""
            hp, hh = divmod(s, 2)
            par = s % 2
            ic, sub = divmod(it, JPAIR)
            av = avps.tile([P, DH + 1], F32, name="av")
            for j in range(JT):
                nc.tensor.matmul(
                    av, e_tiles[par][j][:, ic, sub * P:(sub + 1) * P], vp[j],
                    start=(j == 0), stop=(j == JT - 1))
            rec = recp.tile([P, 1], F32, name="rec")
            nc.vector.reciprocal(out=rec, in_=av[:, DH:DH + 1])
            nc.vector.tensor_scalar(
                out=ao_sb[hp][:, it, hh * DH:(hh + 1) * DH],
                in0=av[:, 0:DH], scalar1=rec, scalar2=None,
                op0=mybir.AluOpType.mult)

        for s in range(HEADS):
            hp, hh = divmod(s, 2)
            par = s % 2
            for jp in range(JT // JPAIR):
                if s == 0 and jp < 2:
                    eb_t = eb_pre[jp]
                else:
                    eb_t = eb_dma(s, jp)
                for jj in range(JPAIR):
                    j = jp * JPAIR + jj
                    s_t = qkps.tile([P, ICH, 512], F32, name="s_t")
                    for ic in range(ICH):
                        nc.tensor.matmul(
                            s_t[:, ic, :],
                            kT2[hh * DH:(hh + 1) * DH, j * P:(j + 1) * P],
                            qT[hp][hh * DH:(hh + 1) * DH,
                                   ic * 512:(ic + 1) * 512],
                            start=True, stop=True,
                            tile_position=(hh * DH, 0))
                    e_t = e_tiles[par][j]
                    nc.scalar.activation(
                        out=e_t, in_=s_t,
                        func=mybir.ActivationFunctionType.Exp)
                    eb_slice = eb_t[:, jj, :].rearrange(
                        "p (c u) -> p c u", c=ICH)
                    nc.vector.tensor_tensor(
                        e_t, e_t, eb_slice, mybir.AluOpType.mult)
                    # previous stage's A@V, one i-block every other j step
                    if s > 0 and j % 2 == 1:
                        av_group(s - 1, j // 2)
        for it in range(ITB):
            av_group(HEADS - 1, it)
        attn_ps.close()

        # ---- phase 4: transpose ao -> [dh, i], project, LayerNorm ----
        fin = ctx.enter_context(tc.tile_pool(name="fin", bufs=3))
        finps = ctx.enter_context(tc.tile_pool(name="finps", bufs=3, space="PSUM"))
        tpps = ctx.enter_context(tc.tile_pool(name="tpps", bufs=3, space="PSUM"))
        for hp in range(NHP):
            for it in range(ITB):
                tp = tpps.tile([P, P], F16, name="tp")
                nc.tensor.transpose(tp, ao_sb[hp][:, it, :], identity)
                nc.vector.tensor_copy(
                    out=aoT[hp][:, it * P:(it + 1) * P], in_=tp)
        for it in range(ITB):
            o_ps = finps.tile([P, DIM], F32, name="o_ps")
            for t in range(CT):
                nc.tensor.matmul(
                    o_ps,
                    aoT[t][:, it * P:(it + 1) * P],
                    wout_sb[t],
                    start=(t == 0), stop=(t == CT - 1))
            stats = fin.tile([P, 6], F32, name="stats")
            nc.vector.bn_stats(out=stats, in_=o_ps)
            mv = fin.tile([P, 2], F32, name="mv")
            nc.vector.bn_aggr(out=mv, in_=stats)
            rstd = fin.tile([P, 1], F32, name="rstd")
            nc.scalar.activation(
                out=rstd, in_=mv[:, 1:2],
                func=mybir.ActivationFunctionType.Sqrt,
                bias=eps_t, scale=1.0)
            nc.vector.reciprocal(out=rstd, in_=rstd)
            negmr = fin.tile([P, 1], F32, name="negmr")
            nc.vector.tensor_scalar(
                out=negmr, in0=mv[:, 0:1], scalar1=rstd, scalar2=-1.0,
                op0=mybir.AluOpType.mult, op1=mybir.AluOpType.mult)
            o_sb = fin.tile([P, DIM], F32, name="o_sb")
            nc.scalar.activation(
                out=o_sb, in_=o_ps,
                func=mybir.ActivationFunctionType.Identity,
                bias=negmr, scale=rstd)
            nc.sync.dma_start(out=out_d[it * P:(it + 1) * P, :], in_=o_sb)


_NC_CACHE = None


def _get_nc():
    global _NC_CACHE
    if _NC_CACHE is None:
        _NC_CACHE = build_bass()
    return _NC_CACHE


def make_in_maps(x, attn_bias, w_q, w_kv, w_out, g_in, g_out):
    x = np.asarray(x, np.float32)
    attn_bias = np.asarray(attn_bias, np.float32)
    g_in = np.asarray(g_in, np.float32)
    wq_eff = np.ascontiguousarray(
        ((g_in[:, None] * np.asarray(w_q, np.float32)) * SCALE).astype(np.float16))
    wkv = g_in[:, None] * np.asarray(w_kv, np.float32)
    # reorder kv projection columns to [v, k]
    wkv_eff = np.ascontiguousarray(
        np.concatenate([wkv[:, DH:], wkv[:, :DH]], axis=1).astype(np.float16))
    w_out = np.ascontiguousarray(
        np.asarray(w_out, np.float32).astype(np.float16))
    np_bias_dt = mybir.dt.np(BIAS_DT)
    biasT = np.exp(np.transpose(attn_bias, (0, 2, 1))).astype(np_bias_dt)  # [h, j, i]
    # bias layout depends only on the query half ih: j rows permuted
    # local-first, then reshaped so each partition line is 16 KiB contiguous:
    # [hp, jp, p, hh, t, i]
    eb_by_ih = []
    for ih in range(2):
        lo, hi = ih * IH, (ih + 1) * IH
        bj = np.concatenate(
            [biasT[:, lo:hi, lo:hi], biasT[:, :lo, lo:hi], biasT[:, hi:, lo:hi]],
            axis=1)  # [h, j, i_local]
        # [h, jp, t, p, i] -> [h, jp, p, t, i] : 8 KiB contiguous per
        # partition line
        b5 = bj.reshape(HEADS, JT // JPAIR, JPAIR, P, IH)
        b5 = np.ascontiguousarray(b5.transpose(0, 1, 3, 2, 4))
        eb_by_ih.append(b5.reshape(HEADS, JT // JPAIR, P, JPAIR * IH))
    in_maps = []
    for c in range(NCORES):
        b, ih = divmod(c, 2)
        lo, hi = ih * IH, (ih + 1) * IH
        xp = np.concatenate([x[b, lo:hi], x[b, :lo], x[b, hi:]], axis=0)
        in_maps.append({
            "x": np.ascontiguousarray(xp),
            "biasT": eb_by_ih[ih],
            "wq": wq_eff, "wkv": wkv_eff, "wout": w_out,
        })
    return in_maps


def assemble(results):
    out = np.empty((B, N, DIM), np.float32)
    for c in range(NCORES):
        b, ih = divmod(c, 2)
        out[b, ih * IH:(ih + 1) * IH, :] = results[c]["out"]
    return out


def kernel(x, attn_bias, w_q, w_kv, w_out, g_in, g_out):
    from concourse.bass_utils import run_bass_kernel_spmd

    in_maps = make_in_maps(x, attn_bias, w_q, w_kv, w_out, g_in, g_out)
    nc = _get_nc()
    res = run_bass_kernel_spmd(nc, in_maps, list(range(NCORES))).results
    return assemble(res) * np.asarray(g_out, np.float32)[None, None, :]


# revision 16
# speedup vs baseline: 1.0440x; 1.0440x over previous
"""Fused LayerNorm->MHA(multi-query)->LayerNorm kernel for TRN2, 8 cores SPMD.

Problem shapes (hardcoded):
  x:        [4, 2048, 512] f32
  attn_bias:[8, 2048, 2048] f32   (shared across batch)
  w_q:      [512, 512], w_kv: [512, 128], w_out: [512, 512]
  g_in, g_out: [512]
  out:      [4, 2048, 512] f32

Sharding: 8 cores = (batch b in 0..3) x (query-half ih in 0..1).
Each core computes the full pipeline for one batch and 1024 query rows.

Restructured v2 (vs the 274us baseline):
  - A@V runs "swapped": the attention tile e[j, i-block] is the PE
    stationary operand and v (64 cols + ones) streams as the moving
    operand -> 65 cols/matmul instead of 512, and the output lands in
    [i, dh] layout with the softmax denominator as a PER-PARTITION
    column, so normalization is one tensor_scalar per tile instead of
    the DRAM scatter/gather reciprocal dance.
  - attn_bias is streamed as exp(bias) fp16 with a host-side layout
    giving 16KiB contiguous HBM lines per partition (128 descriptors
    per 2MiB chunk instead of 512 per 1MiB) -> DMA runs at wire speed.
  - attention emission is interleaved with the tail of LayerNorm /
    projections for j rows 1024:2048 so compute starts after only half
    of phase 1, and the final out-projection consumes PE-transposed
    [dh, i] tiles built in phase 4.
"""

import sys

sys.path.insert(0, "/opt/trn_rl_repo")

import numpy as np
from contextlib import ExitStack

import concourse.bass as bass
import concourse.tile as tile
from concourse import bacc
from concourse import mybir
from concourse.masks import make_identity

B, N, DIM = 4, 2048, 512
HEADS, DH = 8, 64
INNER = HEADS * DH  # 512
EPS = 1e-5
SCALE = DH ** -0.5
NCORES = 8
IH = N // 2  # 1024 query rows per core
P = 128

NT = N // P      # 16 row tiles of x / j tiles
DT = DIM // P    # 4 d tiles
CT = INNER // P  # 4 c tiles (head pairs)
ICH = IH // 512  # 2 i chunks of 512
JT = N // P      # 16 j tiles
JPAIR = 4        # j tiles per bias DMA chunk
NHP = HEADS // 2
ITB = IH // P    # 8 i blocks of 128

F32 = mybir.dt.float32
F16 = mybir.dt.float16

BIAS_DT = mybir.dt.float16


def build_bass():
    nc = bacc.Bacc("TRN2")
    x_d = nc.dram_tensor("x", [N, DIM], F32, kind="ExternalInput")
    # [h, jp, p, t*IH] : per (h, jp) chunk each partition reads one
    # contiguous 8 KiB line
    bias_d = nc.dram_tensor(
        "biasT", [HEADS, JT // JPAIR, P, JPAIR * IH], BIAS_DT,
        kind="ExternalInput")
    wq_d = nc.dram_tensor("wq", [DIM, INNER], F16, kind="ExternalInput")
    wkv_d = nc.dram_tensor("wkv", [DIM, 2 * DH], F16, kind="ExternalInput")
    wout_d = nc.dram_tensor("wout", [INNER, DIM], F16, kind="ExternalInput")
    out_d = nc.dram_tensor("out", [IH, DIM], F32, kind="ExternalOutput")

    with tile.TileContext(nc) as tc:
        _body(tc, x_d, bias_d, wq_d, wkv_d, wout_d, out_d)
    nc.compile()
    return nc


def _body(tc, x_d, bias_d, wq_d, wkv_d, wout_d, out_d):
    nc = tc.nc
    ctx = ExitStack()
    with ctx:
        persist = ctx.enter_context(tc.tile_pool(name="persist", bufs=1))

        identity = persist.tile([P, P], F16, name="identity")
        eps_t = persist.tile([P, 1], F32, name="eps")

        # weights
        wq_sb = [persist.tile([P, INNER], F16, name=f"wq{d}") for d in range(DT)]
        wkv_sb = [persist.tile([P, 2 * DH], F16, name=f"wkv{d}") for d in range(DT)]
        wout_sb = [persist.tile([P, DIM], F16, name=f"wout{t}")
                   for t in range(CT)]

        ebp = ctx.enter_context(tc.tile_pool(name="bias", bufs=3))

        def eb_dma(h, jp):
            t = ebp.tile([P, JPAIR, IH], BIAS_DT, name="eb")
            nc.sync.dma_start(
                out=t,
                in_=bias_d[h, jp].rearrange("p (t i) -> p t i", t=JPAIR))
            return t

        # persistent on-chip tensors
        xnT = [persist.tile([P, N], F16, name=f"xnT{d}") for d in range(DT)]
        kT2 = persist.tile([P, N], F16, name="kT2")
        vp = [persist.tile([P, DH + 1], F16, name=f"vp{j}") for j in range(JT)]
        qT = [persist.tile([P, IH], F16, name=f"qT{t}") for t in range(CT)]
        # attention output, normalized, [i-part, (it, hh*dh)] per head pair
        ao_sb = [persist.tile([P, ITB, P], F16, name=f"ao{t}") for t in range(NHP)]
        # transposed [(hh, dh), i] per head pair
        aoT = [persist.tile([P, IH], F16, name=f"aoT{t}") for t in range(NHP)]

        ln = ctx.enter_context(tc.tile_pool(name="ln", bufs=3))
        proj = ctx.enter_context(tc.tile_pool(name="proj", bufs=2))
        lnps_box = [None]
        projps_box = [None]
        x_tiles_box = [None]

        def ln_group(g, apply_eng):
            lnps = lnps_box[0]
            x_tiles = x_tiles_box[0]
            """LayerNorm 4 x-tiles of group g and transpose into xnT."""
            for nt in range(4 * g, 4 * g + 4):
                x_t = x_tiles[nt]
                stats = ln.tile([P, 6], F32, name="stats")
                nc.vector.bn_stats(out=stats, in_=x_t)
                mv = ln.tile([P, 2], F32, name="mv")
                nc.vector.bn_aggr(out=mv, in_=stats)
                rstd = ln.tile([P, 1], F32, name="rstd")
                nc.scalar.activation(
                    out=rstd, in_=mv[:, 1:2],
                    func=mybir.ActivationFunctionType.Sqrt,
                    bias=eps_t, scale=1.0)
                nc.vector.reciprocal(out=rstd, in_=rstd)
                negmr = ln.tile([P, 1], F32, name="negmr")
                nc.vector.tensor_scalar(
                    out=negmr, in0=mv[:, 0:1], scalar1=rstd, scalar2=-1.0,
                    op0=mybir.AluOpType.mult, op1=mybir.AluOpType.mult)
                xn_t = ln.tile([P, DIM], F16, name="xn_t")
                if apply_eng == "act":
                    nc.scalar.activation(
                        out=xn_t, in_=x_t,
                        func=mybir.ActivationFunctionType.Identity,
                        bias=negmr, scale=rstd)
                else:
                    nc.vector.tensor_scalar(
                        out=xn_t, in0=x_t, scalar1=rstd, scalar2=negmr,
                        op0=mybir.AluOpType.mult, op1=mybir.AluOpType.add)
                for d in range(DT):
                    ps = lnps.tile([P, P], F16, name="tps")
                    nc.tensor.transpose(ps, xn_t[:, d * P:(d + 1) * P], identity)
                    nc.vector.tensor_copy(
                        out=xnT[d][:, nt * P:(nt + 1) * P], in_=ps)

        def kv_group(g):
            """kv projection + kT2 + v row tiles for j chunk g (512 rows)."""
            projps = projps_box[0]
            lo = g * 512
            kvT = proj.tile([P, 512], F16, name="kvT")
            ps = projps.tile([P, 512], F32, name="kvps")
            for d in range(DT):
                nc.tensor.matmul(
                    ps, wkv_sb[d], xnT[d][:, lo:lo + 512],
                    start=(d == 0), stop=(d == DT - 1))
            nc.vector.tensor_copy(out=kvT, in_=ps)
            # k rows (partitions 64:128, host-swapped [v, k]) into both kT2
            # halves; partition-shifting copy goes through DMA
            nc.vector.tensor_copy(out=kT2[DH:2 * DH, lo:lo + 512],
                                  in_=kvT[DH:2 * DH, :])
            nc.sync.dma_start(out=kT2[0:DH, lo:lo + 512], in_=kvT[DH:2 * DH, :])
            # v row tiles with ones column
            for jj in range(4):
                j = 4 * g + jj
                vps = projps.tile([P, DH], F16, name="vps")
                nc.tensor.transpose(
                    vps, kvT[0:DH, jj * P:(jj + 1) * P], identity[0:DH, 0:DH])
                nc.vector.tensor_copy(out=vp[j][:, 0:DH], in_=vps)
                nc.vector.memset(vp[j][:, DH:DH + 1], 1.0)

        def q_group(g):
            """q projection for i chunk g (512 rows)."""
            projps = projps_box[0]
            lo = g * 512
            for t in range(CT):
                ps = projps.tile([P, 512], F32, name="qps")
                for d in range(DT):
                    nc.tensor.matmul(
                        ps, wq_sb[d][:, t * P:(t + 1) * P],
                        xnT[d][:, lo:lo + 512],
                        start=(d == 0), stop=(d == DT - 1))
                nc.vector.tensor_copy(out=qT[t][:, lo:lo + 512], in_=ps)

        # Phase 1+2: LayerNorm + projections. Local query rows (0:1024)
        # first; bias chunks for the first head interleave onto the DMA ring
        # before x groups 2-3 so attention can start as soon as qT is ready.
        eb_pre = []
        with tc.tile_pool(name="xload", bufs=1) as xload, \
             tc.tile_pool(name="lnps", bufs=2, space="PSUM") as lnps, \
             tc.tile_pool(name="projps", bufs=2, space="PSUM") as projps:
            x_big = [xload.tile([P, 4, DIM], F32, name=f"x{g}")
                     for g in range(4)]
            for g in range(2):
                nc.sync.dma_start(
                    out=x_big[g],
                    in_=x_d[g * 4 * P:(g + 1) * 4 * P, :]
                    .rearrange("(t p) d -> p t d", p=P))
            make_identity(nc, identity)
            nc.vector.memset(eps_t, EPS)
            for d in range(DT):
                nc.sync.dma_start(out=wq_sb[d], in_=wq_d[d * P:(d + 1) * P, :])
                nc.sync.dma_start(out=wkv_sb[d], in_=wkv_d[d * P:(d + 1) * P, :])
            for t in range(CT):
                nc.sync.dma_start(out=wout_sb[t],
                                  in_=wout_d[t * P:(t + 1) * P, :])
            for g in range(2, 4):
                nc.sync.dma_start(
                    out=x_big[g],
                    in_=x_d[g * 4 * P:(g + 1) * 4 * P, :]
                    .rearrange("(t p) d -> p t d", p=P))
            eb_pre.append(eb_dma(0, 0))
            eb_pre.append(eb_dma(0, 1))
            x_tiles_box[0] = [x_big[nt // 4][:, nt % 4, :] for nt in range(NT)]
            lnps_box[0] = lnps
            projps_box[0] = projps
            for g in range(2):
                ln_group(g, "act")
                kv_group(g)
                q_group(g)
            for g in range(2, 4):
                ln_group(g, "dve")
                kv_group(g)

        # ---- attention: 8 stages (head-serial); A@V runs pipelined one
        # stage behind so each accumulation group owns one PSUM bank ----
        ep = ctx.enter_context(tc.tile_pool(name="ep", bufs=1))
        recp = ctx.enter_context(tc.tile_pool(name="recp", bufs=4))
        e_tiles = [[ep.tile([P, ICH, 512], F16, name=f"e{par}_{j}")
                    for j in range(JT)] for par in range(2)]
        attn_ps = ExitStack()
        qkps = attn_ps.enter_context(
            tc.tile_pool(name="qkps", bufs=2, space="PSUM"))
        avps = attn_ps.enter_context(
            tc.tile_pool(name="avps", bufs=3, space="PSUM"))

        def av_group(s, it):
            """A@V for stage s (head s), i block it: one 16-matmul
            accumulation group in a single PSUM bank, then normalize."""
            hp, hh = divmod(s, 2)
            par = s % 2
            ic, sub = divmod(it, JPAIR)
            av = avps.tile([P, DH + 1], F32, name="av")
            for j in range(JT):
                nc.tensor.matmul(
                    av, e_tiles[par][j][:, ic, sub * P:(sub + 1) * P], vp[j],
                    start=(j == 0), stop=(j == JT - 1))
            rec = recp.tile([P, 1], F32, name="rec")
            nc.vector.reciprocal(out=rec, in_=av[:, DH:DH + 1])
            nc.vector.tensor_scalar(
                out=ao_sb[hp][:, it, hh * DH:(hh + 1) * DH],
                in0=av[:, 0:DH], scalar1=rec, scalar2=None,
                op0=mybir.AluOpType.mult)

        for s in range(HEADS):
            hp, hh = divmod(s, 2)
            par = s % 2
            for jp in range(JT // JPAIR):
                if s == 0 and jp < 2:
                    eb_t = eb_pre[jp]
                else:
                    eb_t = eb_dma(s, jp)
                for jj in range(JPAIR):
                    j = jp * JPAIR + jj
                    s_t = qkps.tile([P, ICH, 512], F32, name="s_t")
                    for ic in range(ICH):
                        nc.tensor.matmul(
                            s_t[:, ic, :],
                            kT2[hh * DH:(hh + 1) * DH, j * P:(j + 1) * P],
                            qT[hp][hh * DH:(hh + 1) * DH,
                                   ic * 512:(ic + 1) * 512],
                            start=True, stop=True,
                            tile_position=(hh * DH, 0))
                    e_t = e_tiles[par][j]
                    nc.scalar.activation(
                        out=e_t, in_=s_t,
                        func=mybir.ActivationFunctionType.Exp)
                    eb_slice = eb_t[:, jj, :].rearrange(
                        "p (c u) -> p c u", c=ICH)
                    nc.vector.tensor_tensor(
                        e_t, e_t, eb_slice, mybir.AluOpType.mult)
                    # previous stage's A@V, one i-block every other j step
                    if s > 0 and j % 2 == 1:
                        av_group(s - 1, j // 2)
        for it in range(ITB):
            av_group(HEADS - 1, it)
        attn_ps.close()

        # ---- phase 4: transpose ao -> [dh, i], project, LayerNorm ----
        fin = ctx.enter_context(tc.tile_pool(name="fin", bufs=3))
        finps = ctx.enter_context(tc.tile_pool(name="finps", bufs=3, space="PSUM"))
        tpps = ctx.enter_context(tc.tile_pool(name="tpps", bufs=3, space="PSUM"))
        for hp in range(NHP):
            for it in range(ITB):
                tp = tpps.tile([P, P], F16, name="tp")
                nc.tensor.transpose(tp, ao_sb[hp][:, it, :], identity)
                nc.vector.tensor_copy(
                    out=aoT[hp][:, it * P:(it + 1) * P], in_=tp)
        for it in range(ITB):
            o_ps = finps.tile([P, DIM], F32, name="o_ps")
            for t in range(CT):
                nc.tensor.matmul(
                    o_ps,
                    aoT[t][:, it * P:(it + 1) * P],
                    wout_sb[t],
                    start=(t == 0), stop=(t == CT - 1))
            stats = fin.tile([P, 6], F32, name="stats")
            nc.vector.bn_stats(out=stats, in_=o_ps)
            mv = fin.tile([P, 2], F32, name="mv")
            nc.vector.bn_aggr(out=mv, in_=stats)
            rstd = fin.tile([P, 1], F32, name="rstd")
            nc.scalar.activation(
                out=rstd, in_=mv[:, 1:2],
                func=mybir.ActivationFunctionType.Sqrt,
                bias=eps_t, scale=1.0)
            nc.vector.reciprocal(out=rstd, in_=rstd)
            negmr = fin.tile([P, 1], F32, name="negmr")
            nc.vector.tensor_scalar(
                out=negmr, in0=mv[:, 0:1], scalar1=rstd, scalar2=-1.0,
                op0=mybir.AluOpType.mult, op1=mybir.AluOpType.mult)
            o_sb = fin.tile([P, DIM], F32, name="o_sb")
            nc.scalar.activation(
                out=o_sb, in_=o_ps,
                func=mybir.ActivationFunctionType.Identity,
                bias=negmr, scale=rstd)
            nc.sync.dma_start(out=out_d[it * P:(it + 1) * P, :], in_=o_sb)


_NC_CACHE = None


def _get_nc():
    global _NC_CACHE
    if _NC_CACHE is None:
        _NC_CACHE = build_bass()
    return _NC_CACHE


def make_in_maps(x, attn_bias, w_q, w_kv, w_out, g_in, g_out):
    x = np.asarray(x, np.float32)
    attn_bias = np.asarray(attn_bias, np.float32)
    g_in = np.asarray(g_in, np.float32)
    wq_eff = np.ascontiguousarray(
        ((g_in[:, None] * np.asarray(w_q, np.float32)) * SCALE).astype(np.float16))
    wkv = g_in[:, None] * np.asarray(w_kv, np.float32)
    # reorder kv projection columns to [v, k]
    wkv_eff = np.ascontiguousarray(
        np.concatenate([wkv[:, DH:], wkv[:, :DH]], axis=1).astype(np.float16))
    w_out = np.ascontiguousarray(
        np.asarray(w_out, np.float32).astype(np.float16))
    np_bias_dt = mybir.dt.np(BIAS_DT)
    biasT = np.exp(np.transpose(attn_bias, (0, 2, 1))).astype(np_bias_dt)  # [h, j, i]
    # bias layout depends only on the query half ih: j rows permuted
    # local-first, then reshaped so each partition line is 16 KiB contiguous:
    # [hp, jp, p, hh, t, i]
    eb_by_ih = []
    for ih in range(2):
        lo, hi = ih * IH, (ih + 1) * IH
        bj = np.concatenate(
            [biasT[:, lo:hi, lo:hi], biasT[:, :lo, lo:hi], biasT[:, hi:, lo:hi]],
            axis=1)  # [h, j, i_local]
        # [h, jp, t, p, i] -> [h, jp, p, t, i] : 8 KiB contiguous per
        # partition line
        b5 = bj.reshape(HEADS, JT // JPAIR, JPAIR, P, IH)
        b5 = np.ascontiguousarray(b5.transpose(0, 1, 3, 2, 4))
        eb_by_ih.append(b5.reshape(HEADS, JT // JPAIR, P, JPAIR * IH))
    in_maps = []
    for c in range(NCORES):
        b, ih = divmod(c, 2)
        lo, hi = ih * IH, (ih + 1) * IH
        xp = np.concatenate([x[b, lo:hi], x[b, :lo], x[b, hi:]], axis=0)
        in_maps.append({
            "x": np.ascontiguousarray(xp),
            "biasT": eb_by_ih[ih],
            "wq": wq_eff, "wkv": wkv_eff, "wout": w_out,
        })
    return in_maps


def assemble(results):
    out = np.empty((B, N, DIM), np.float32)
    for c in range(NCORES):
        b, ih = divmod(c, 2)
        out[b, ih * IH:(ih + 1) * IH, :] = results[c]["out"]
    return out


def kernel(x, attn_bias, w_q, w_kv, w_out, g_in, g_out):
    from concourse.bass_utils import run_bass_kernel_spmd

    in_maps = make_in_maps(x, attn_bias, w_q, w_kv, w_out, g_in, g_out)
    nc = _get_nc()
    res = run_bass_kernel_spmd(nc, in_maps, list(range(NCORES))).results
    return assemble(res) * np.asarray(g_out, np.float32)[None, None, :]
